# revision 36
# baseline (speedup 1.0000x reference)
"""Trainium2 Bass kernel: MultiHeadAttention + residual + LayerNorm.

Problem shapes (hardcoded):
  x: (2, 2048, 1024) f32, 16 heads x 64 head_dim, scale = 64**-0.5
  y = LayerNorm(x + MHA(x))

Sharding: token-parallel over 8 cores. Core c handles batch b=c//4 and
query tokens [512*(c%4), 512*(c%4+1)) of that batch. Each core receives
its batch's full token sequence ROTATED so that its own 512 query tokens
are rows 0..511 (attention is permutation-invariant over keys, so K/V
token order does not matter). No cross-core collectives needed.
"""

import sys

sys.path.insert(0, "/opt/trn_rl_repo")

import numpy as np

import concourse.bass as bass
import concourse.bacc as bacc
import concourse.mybir as mybir
import concourse.tile as tile
from concourse import bass_utils
from concourse.masks import make_identity

# ---- problem constants ----
B = 2
S = 2048
D = 1024
H = 16
DH = 64
SCALE = DH ** -0.5
EPS = 1e-5

N_CORES = 8
CORES_PER_BATCH = N_CORES // B
TQ = S // CORES_PER_BATCH          # 512 query tokens per core
NT = S // 128                      # 16 key tiles of 128
ND = D // 128                      # 8 dim tiles of 128
NPAIR = H // 2                     # 8 head pairs
NTQ = TQ // 128                    # 4 query tiles

F32 = mybir.dt.float32
F32R = mybir.dt.float32r
BF16 = mybir.dt.bfloat16

# matmul compute dtype: "f32" (exact, 4 cyc/row), "f32r" (1 cyc/row, relaxed),
# "bf16" (1 cyc/row, bf16 storage)
MM_MODE = "f32r"


def _build_program(mm_mode: str):
    """Build the SPMD Bass program (same for all 8 cores)."""
    nc = bacc.Bacc("TRN2", target_bir_lowering=False, debug=False,
                   num_devices=N_CORES)

    # storage dtype for matmul operand tiles. float32r / bf16 rounding is
    # applied by the compute op that writes each tile (PSUM->SBUF copies).
    sdt = {"f32": F32, "f32r": F32R, "bf16": BF16}[mm_mode]

    def mm(ap):
        return ap

    # ---- DRAM I/O ----
    # x host-pretransposed: xbT[p, d, t] = x[t, 128d+p]; xq = x rows 0..TQ
    xbT_d = nc.dram_tensor("xbT", (128, ND, S), F32, kind="ExternalInput").ap()
    xq_d = nc.dram_tensor("xq", (TQ, D), F32, kind="ExternalInput").ap()
    # weights host-packed: wX[p, otile, dtile, c] = WX[128*dtile+p, 128*otile+c]
    wq_d = nc.dram_tensor("wq", (128, ND, ND, 128), F32,
                          kind="ExternalInput").ap()
    wk_d = nc.dram_tensor("wk", (128, ND, ND, 128), F32,
                          kind="ExternalInput").ap()
    wv_d = nc.dram_tensor("wv", (128, ND, ND, 128), F32,
                          kind="ExternalInput").ap()
    # wo[p, dtile, o] = Wo[128*dtile+p, o]
    wo_d = nc.dram_tensor("wo", (128, ND, D), F32, kind="ExternalInput").ap()
    # biases host-packed [p, otile]
    bq_d = nc.dram_tensor("bq", (128, ND), F32, kind="ExternalInput").ap()
    bk_d = nc.dram_tensor("bk", (128, ND), F32, kind="ExternalInput").ap()
    bv_d = nc.dram_tensor("bv", (128, ND), F32, kind="ExternalInput").ap()
    bo_d = nc.dram_tensor("bo", (D,), F32, kind="ExternalInput").ap()
    gamma_d = nc.dram_tensor("gamma", (D,), F32, kind="ExternalInput").ap()
    beta_d = nc.dram_tensor("beta", (D,), F32, kind="ExternalInput").ap()
    y_d = nc.dram_tensor("y", (TQ, D), F32, kind="ExternalOutput").ap()

    def bcast_rows(src_row_ap, nrows):
        # replicate a [1, N] AP across nrows partitions (DMA only)
        return bass.AP(tensor=src_row_ap.tensor, offset=src_row_ap.offset,
                       ap=[[0, nrows]] + [list(d) for d in src_row_ap.ap[-1:]])

    with tile.TileContext(nc) as tc:
        from contextlib import ExitStack
        with ExitStack() as ctx:
            # ---- pools ----
            consts = ctx.enter_context(tc.tile_pool(name="consts", bufs=1))
            bigp = ctx.enter_context(tc.tile_pool(name="big", bufs=1))
            wslice = ctx.enter_context(tc.tile_pool(name="wslice", bufs=2))
            ktp = ctx.enter_context(tc.tile_pool(name="ktp", bufs=1))
            vts = ctx.enter_context(tc.tile_pool(name="vts", bufs=1))
            vaug = ctx.enter_context(tc.tile_pool(name="vaug", bufs=2))
            expp = ctx.enter_context(tc.tile_pool(name="expp", bufs=4))
            smallp = ctx.enter_context(tc.tile_pool(name="small", bufs=2))
            xnatp = ctx.enter_context(tc.tile_pool(name="xnat", bufs=2))
            ybufp = ctx.enter_context(tc.tile_pool(name="ybuf", bufs=1))

            ps_tr = ctx.enter_context(
                tc.tile_pool(name="ps_tr", bufs=2, space="PSUM"))
            ps_acc = ctx.enter_context(
                tc.tile_pool(name="ps_acc", bufs=2, space="PSUM"))
            ps_sc = ctx.enter_context(
                tc.tile_pool(name="ps_sc", bufs=2, space="PSUM"))

            # ---- constants ----
            ident = xnatp.tile([128, 128], F32, tag="xnat", name="ident")
            make_identity(nc, ident)
            eps_t = consts.tile([128, 1], F32)
            nc.vector.memset(eps_t, EPS)
            ones1 = consts.tile([128, 64], F32)
            nc.vector.memset(ones1, 1.0)
            rdt = F32 if sdt == F32 else F32R
            ones_r = consts.tile([128, 64], rdt)
            nc.vector.tensor_copy(out=ones_r, in_=ones1)
            ident_s = consts.tile([128, 128], sdt)
            nc.vector.tensor_copy(out=ident_s, in_=ident)
            ones_nt = consts.tile([128, NT, 2], F32)
            nc.vector.memset(ones_nt, 1.0)
            # per-partition biases [128, ND]: column j is bias[128j:128j+128]
            bq_t = consts.tile([128, ND], F32)
            nc.sync.dma_start(out=bq_t, in_=bq_d)
            bk_t = consts.tile([128, ND], F32)
            nc.sync.dma_start(out=bk_t, in_=bk_d)
            bv_t = consts.tile([128, ND], F32)
            nc.sync.dma_start(out=bv_t, in_=bv_d)

            # =========================================================
            # Phase A: load xT[p, d, t] = x[t, 128d+p] (pre-transposed on host)
            # =========================================================
            xT = bigp.tile([128, ND, S], sdt, tag="xT")
            xTd = [xT[:, d, :] for d in range(ND)]
            if sdt != F32:
                for d in range(ND):
                    for hf in range(2):
                        xtf = xnatp.tile([128, S // 2], F32, tag="xnat")
                        sl = slice(hf * (S // 2), (hf + 1) * (S // 2))
                        nc.sync.dma_start(out=xtf, in_=xbT_d[:, d, sl])
                        nc.vector.tensor_copy(out=xT[:, d, sl], in_=xtf)
            else:
                nc.sync.dma_start(out=xT, in_=xbT_d)

            # =========================================================
            # Phase B: QT[p, j, tq] = q[tq, 128j+p] for own tokens 0..TQ
            #   q = x @ Wq + bq   (scale folded into exp later)
            # =========================================================
            qT = bigp.tile([128, ND, TQ], sdt, tag="qT")
            for j in range(ND):
                wq_s = wslice.tile([128, ND, 128], sdt, tag="wsl")
                src = wq_d[:, j, :, :]
                if sdt != F32:
                    wq_f = xnatp.tile([128, ND, 128], F32, tag="xnat")
                    nc.sync.dma_start(out=wq_f, in_=src)
                    nc.vector.tensor_copy(out=wq_s, in_=wq_f)
                else:
                    nc.sync.dma_start(out=wq_s, in_=src)
                pq = ps_acc.tile([128, TQ], F32, tag="acc")
                for d in range(ND):
                    nc.tensor.matmul(pq, mm(wq_s[:, d, :]), mm(xTd[d][:, 0:TQ]),
                                     start=(d == 0), stop=(d == ND - 1))
                nc.vector.tensor_scalar_add(
                    out=qT[:, j, :], in0=pq, scalar1=bq_t[:, j:j + 1])

            # =========================================================
            # Phase C: per head-pair projections + attention
            # =========================================================
            outT = bigp.tile([128, ND, TQ], sdt, tag="outT")

            for p in range(NPAIR):
                # -- W slices for this pair --
                wk_s = wslice.tile([128, ND, 128], sdt, tag="wsl")
                wv_s = wslice.tile([128, ND, 128], sdt, tag="wsl")
                ksrc = wk_d[:, p, :, :]
                vsrc = wv_d[:, p, :, :]
                if sdt != F32:
                    wk_f = xnatp.tile([128, ND, 128], F32, tag="xnat")
                    wv_f = xnatp.tile([128, ND, 128], F32, tag="xnat")
                    nc.sync.dma_start(out=wk_f, in_=ksrc)
                    nc.sync.dma_start(out=wv_f, in_=vsrc)
                    nc.vector.tensor_copy(out=wk_s, in_=wk_f)
                    nc.vector.tensor_copy(out=wv_s, in_=wv_f)
                else:
                    nc.sync.dma_start(out=wk_s, in_=ksrc)
                    nc.sync.dma_start(out=wv_s, in_=vsrc)

                # -- K^T for pair: [128(dh pair), NT*128] --
                kT = ktp.tile([128, NT // 4, 512], sdt, tag="kT")
                for n in range(NT // 4):
                    pk = ps_acc.tile([128, 512], F32, tag="acc")
                    for d in range(ND):
                        nc.tensor.matmul(
                            pk, mm(wk_s[:, d, :]),
                            mm(xTd[d][:, 512 * n:512 * (n + 1)]),
                            start=(d == 0), stop=(d == ND - 1))
                    nc.vector.tensor_scalar_add(
                        out=kT[:, n, :], in0=pk, scalar1=bk_t[:, p:p + 1])

                # -- V for pair, via V^T then PE transpose, into V_aug --
                # V_aug[:, t, 65*he : 65*he+65] = [V_head | ones]
                va = vaug.tile([128, NT, 130], sdt, tag="va")
                va_ones = bass.AP(tensor=va.tensor, offset=va.offset + 64,
                                  ap=[list(va.ap[0]), [130, NT], [65, 2]])
                nc.vector.tensor_copy(out=va_ones, in_=ones_nt)
                for n in range(NT // 4):
                    pv = ps_acc.tile([128, 512], F32, tag="acc")
                    for d in range(ND):
                        nc.tensor.matmul(
                            pv, mm(wv_s[:, d, :]),
                            mm(xTd[d][:, 512 * n:512 * (n + 1)]),
                            start=(d == 0), stop=(d == ND - 1))
                    vts_t = vts.tile([128, 512], sdt, tag="vts")
                    nc.vector.tensor_scalar_add(
                        out=vts_t, in0=pv, scalar1=bv_t[:, p:p + 1])
                    for s in range(4):
                        t = 4 * n + s
                        pt = ps_tr.tile([128, 128], sdt, tag="tr")
                        nc.tensor.transpose(
                            pt, vts_t[:, 128 * s:128 * (s + 1)], ident_s)
                        va_v = bass.AP(
                            tensor=va.tensor, offset=va.offset + 130 * t,
                            ap=[list(va.ap[0]), [65, 2], [1, 64]])
                        pt_v = bass.AP(tensor=pt.tensor, offset=pt.offset,
                                       ap=[list(pt.ap[0]), [64, 2], [1, 64]])
                        nc.vector.tensor_copy(out=va_v, in_=pt_v)

                # -- attention: both heads interleaved in quarter-chunks so
                # PE always has matmuls queued while ScalarE runs exp --
                NCHUNK = 4
                TPC = NT // NCHUNK  # key tiles per chunk
                pav = [None, None]
                for he in range(2):
                    pav[he] = ps_acc.tile([128, TQ], F32, tag="acc",
                                          name=f"pav{he}")
                exq = {}
                for ch in range(NCHUNK):
                    for he in range(2):
                        ex = expp.tile([128, TPC, TQ], sdt, tag="ex",
                                       name=f"ex{he}_{ch}")
                        exq[(he, ch)] = ex
                        for g in range(TPC // 2):
                            psc = ps_sc.tile([128, 2, TQ], F32, tag="sc",
                                             name="psc")
                            for s2 in range(2):
                                t = ch * TPC + 2 * g + s2
                                lhs = kT[64 * he:64 * (he + 1),
                                         t // 4,
                                         128 * (t % 4):128 * (t % 4 + 1)]
                                rhs = qT[64 * he:64 * (he + 1), p, :]
                                nc.tensor.matmul(psc[:, s2, :], mm(lhs),
                                                 mm(rhs), start=True,
                                                 stop=True)
                            nc.scalar.activation(
                                out=ex[:, 2 * g:2 * g + 2, :], in_=psc,
                                func=mybir.ActivationFunctionType.Exp,
                                scale=SCALE)
                    for he in range(2):
                        ex = exq[(he, ch)]
                        for tt in range(TPC):
                            t = ch * TPC + tt
                            lhs = va[:, t, 65 * he:65 * he + 65]
                            nc.tensor.matmul(
                                pav[he][0:65, :], mm(lhs), mm(ex[:, tt, :]),
                                start=(t == 0), stop=(t == NT - 1))
                for he in range(2):
                    # normalize: out^T / denom. Broadcast the RAW denominator
                    # row across 64 partitions via a K=1 outer-product matmul
                    # (keeps the slow divide off the PE critical path), then
                    # one DVE divide.
                    dns = smallp.tile([128, TQ], rdt, tag="rcp", name="dns")
                    nc.vector.tensor_copy(out=dns[64:65, :],
                                          in_=pav[he][64:65, :])
                    rb = ps_tr.tile([64, TQ], F32, tag="tr", name="rb")
                    nc.tensor.matmul(rb, ones_r[64:65, :], dns[64:65, :],
                                     start=True, stop=True)
                    scr = smallp.tile([64, TQ], F32, tag="rbs", name="scr")
                    rrec = smallp.tile([64, TQ], F32, tag="rrec", name="rrec")
                    nc.vector.reciprocal_approx_accurate(
                        out=rrec, in_=rb, scratch=scr)
                    if he == 0:
                        nc.vector.tensor_mul(
                            out=outT[0:64, p, :],
                            in0=pav[he][0:64, :], in1=rrec[0:64, :])
                    else:
                        # compute at partitions 0:64, then DMA-shift to 64:128
                        tmp = smallp.tile([128, TQ], sdt, tag="otmp",
                                          name="tmp")
                        nc.vector.tensor_mul(
                            out=tmp[0:64, :],
                            in0=pav[he][0:64, :], in1=rrec[0:64, :])
                        nc.gpsimd.dma_start(
                            out=outT[64:128, p, :], in_=tmp[0:64, :])

            # =========================================================
            # Phase D: out-proj + residual + LayerNorm
            # =========================================================
            # bo/gamma/beta broadcast rows land in a dead expS slot
            lnc = expp.tile([128, 3, D], F32, tag="ex", name="lnc")
            nc.gpsimd.dma_start(out=lnc[:, 0, :], in_=bcast_rows(bo_d[None], 128))
            nc.gpsimd.dma_start(out=lnc[:, 1, :],
                                in_=bcast_rows(gamma_d[None], 128))
            nc.gpsimd.dma_start(out=lnc[:, 2, :],
                                in_=bcast_rows(beta_d[None], 128))
            bo_b, gamma_b, beta_b = lnc[:, 0, :], lnc[:, 1, :], lnc[:, 2, :]
            wo_t = bigp.tile([128, ND, D], sdt, tag="xT")  # reuse xT slot
            if sdt != F32:
                for d in range(ND):
                    wo_f = xnatp.tile([128, D], F32, tag="xnat")
                    nc.sync.dma_start(out=wo_f, in_=wo_d[:, d, :])
                    nc.vector.tensor_copy(out=wo_t[:, d, :], in_=wo_f)
            else:
                nc.sync.dma_start(out=wo_t, in_=wo_d)
            xq_t = bigp.tile([128, NTQ, D], F32, tag="qT")  # reuse qT slot
            nc.sync.dma_start(
                out=xq_t, in_=xq_d.rearrange("(i p) d -> p i d", p=128))

            for i in range(NTQ):
                po = ps_sc.tile([128, 2, 512], F32, tag="sc", name="po")
                for half in range(2):
                    dst = po[:, half, :]
                    for d in range(ND):
                        nc.tensor.matmul(
                            dst, mm(outT[:, d, 128 * i:128 * (i + 1)]),
                            mm(wo_t[:, d, 512 * half:512 * (half + 1)]),
                            start=(d == 0), stop=(d == ND - 1))
                ysb = ybufp.tile([128, D], F32, tag="ysb")
                pflat = po.rearrange("p a b -> p (a b)")
                # y = out + bo + x
                nc.vector.tensor_add(out=ysb, in0=pflat, in1=bo_b)
                nc.vector.tensor_add(out=ysb, in0=ysb, in1=xq_t[:, i, :])
                # LayerNorm
                stats = smallp.tile([128, 2, 6], F32, tag="stats")
                mv = smallp.tile([128, 2], F32, tag="mv")
                yv = ysb.rearrange("p (a b) -> p a b", a=2)
                for sg in range(2):
                    nc.vector.bn_stats(out=stats[:, sg, :], in_=yv[:, sg, :])
                nc.vector.bn_aggr(out=mv, in_=stats)
                sd = smallp.tile([128, 1], F32, tag="sd")
                nc.scalar.activation(out=sd, in_=mv[:, 1:2],
                                     func=mybir.ActivationFunctionType.Sqrt,
                                     bias=eps_t, scale=1.0)
                rstd = smallp.tile([128, 1], F32, tag="rstd")
                nc.vector.reciprocal(out=rstd, in_=sd)
                nc.vector.tensor_scalar(
                    out=ysb, in0=ysb, scalar1=mv[:, 0:1], scalar2=rstd,
                    op0=mybir.AluOpType.subtract, op1=mybir.AluOpType.mult)
                nc.vector.tensor_mul(out=ysb, in0=ysb, in1=gamma_b)
                nc.vector.tensor_add(out=ysb, in0=ysb, in1=beta_b)
                nc.sync.dma_start(out=y_d[128 * i:128 * (i + 1), :], in_=ysb)

    nc.compile()
    return nc


_PROGRAM_CACHE = {}


def _get_program(mm_mode: str):
    if mm_mode not in _PROGRAM_CACHE:
        _PROGRAM_CACHE[mm_mode] = _build_program(mm_mode)
    return _PROGRAM_CACHE[mm_mode]


def _pack_w(w):
    # [p, otile, dtile, c] = W[128*dtile+p, 128*otile+c], contiguous
    w = np.asarray(w, np.float32).reshape(ND, 128, ND, 128)
    return np.ascontiguousarray(w.transpose(1, 2, 0, 3))


def _pack_wo(w):
    # [p, dtile, o] = W[128*dtile+p, o]
    w = np.asarray(w, np.float32).reshape(ND, 128, D)
    return np.ascontiguousarray(w.transpose(1, 0, 2))


def _pack_b(b):
    # [p, otile] = b[128*otile+p]
    b = np.asarray(b, np.float32).reshape(ND, 128)
    return np.ascontiguousarray(b.transpose(1, 0))


def kernel(x, Wq, bq, Wk, bk, Wv, bv, Wo, bo, gamma, beta, _trace=False):
    x = np.asarray(x, dtype=np.float32)
    nc = _get_program(MM_MODE)

    wq_p, wk_p, wv_p = _pack_w(Wq), _pack_w(Wk), _pack_w(Wv)
    wo_p = _pack_wo(Wo)
    bq_p, bk_p, bv_p = _pack_b(bq), _pack_b(bk), _pack_b(bv)
    in_maps = []
    for c in range(N_CORES):
        b = c // CORES_PER_BATCH
        off = TQ * (c % CORES_PER_BATCH)
        xb = np.concatenate([x[b, off:], x[b, :off]], axis=0)
        xbT = np.ascontiguousarray(
            xb.T.reshape(ND, 128, S).transpose(1, 0, 2))
        in_maps.append({
            "xbT": xbT,
            "xq": np.ascontiguousarray(xb[0:TQ]),
            "wq": wq_p, "wk": wk_p, "wv": wv_p, "wo": wo_p,
            "bq": bq_p, "bk": bk_p, "bv": bv_p,
            "bo": np.asarray(bo, np.float32),
            "gamma": np.asarray(gamma, np.float32),
            "beta": np.asarray(beta, np.float32),
        })

    res = bass_utils.run_bass_kernel_spmd(
        nc, in_maps, list(range(N_CORES)), trace=_trace)

    y = np.empty((B, S, D), dtype=np.float32)
    for c in range(N_CORES):
        b = c // CORES_PER_BATCH
        off = TQ * (c % CORES_PER_BATCH)
        y[b, off:off + TQ] = res.results[c]["y"]

    kernel.last_exec_time_ns = res.exec_time_ns
    return y


kernel.last_exec_time_ns = None


# revision 37
# speedup vs baseline: 1.0671x; 1.0671x over previous
"""Trainium2 Bass kernel: MultiHeadAttention + residual + LayerNorm.

Problem shapes (hardcoded):
  x: (2, 2048, 1024) f32, 16 heads x 64 head_dim, scale = 64**-0.5
  y = LayerNorm(x + MHA(x))

Sharding: token-parallel over 8 cores. Core c handles batch b=c//4 and
query tokens [512*(c%4), 512*(c%4+1)) of that batch. Each core receives
its batch's full token sequence ROTATED so that its own 512 query tokens
are rows 0..511 (attention is permutation-invariant over keys, so K/V
token order does not matter). No cross-core collectives needed.
"""

import sys

sys.path.insert(0, "/opt/trn_rl_repo")

import numpy as np

import concourse.bass as bass
import concourse.bacc as bacc
import concourse.mybir as mybir
import concourse.tile as tile
from concourse import bass_utils
from concourse.masks import make_identity

# ---- problem constants ----
B = 2
S = 2048
D = 1024
H = 16
DH = 64
SCALE = DH ** -0.5
EPS = 1e-5

N_CORES = 8
CORES_PER_BATCH = N_CORES // B
TQ = S // CORES_PER_BATCH          # 512 query tokens per core
NT = S // 128                      # 16 key tiles of 128
ND = D // 128                      # 8 dim tiles of 128
NPAIR = H // 2                     # 8 head pairs
NTQ = TQ // 128                    # 4 query tiles

F32 = mybir.dt.float32
F32R = mybir.dt.float32r
BF16 = mybir.dt.bfloat16

# matmul compute dtype: "f32" (exact, 4 cyc/row), "f32r" (1 cyc/row, relaxed),
# "bf16" (1 cyc/row, bf16 storage)
MM_MODE = "f32r"


def _build_program(mm_mode: str):
    """Build the SPMD Bass program (same for all 8 cores)."""
    nc = bacc.Bacc("TRN2", target_bir_lowering=False, debug=False,
                   num_devices=N_CORES)

    # storage dtype for matmul operand tiles. float32r / bf16 rounding is
    # applied by the compute op that writes each tile (PSUM->SBUF copies).
    sdt = {"f32": F32, "f32r": F32R, "bf16": BF16}[mm_mode]

    def mm(ap):
        return ap

    # ---- DRAM I/O ----
    # x host-pretransposed: xbT[p, d, t] = x[t, 128d+p]; xq = x rows 0..TQ
    xbT_d = nc.dram_tensor("xbT", (128, ND, S), F32, kind="ExternalInput").ap()
    xq_d = nc.dram_tensor("xq", (TQ, D), F32, kind="ExternalInput").ap()
    # weights host-packed: wX[p, otile, dtile, c] = WX[128*dtile+p, 128*otile+c]
    wq_d = nc.dram_tensor("wq", (128, ND, ND, 128), F32,
                          kind="ExternalInput").ap()
    wk_d = nc.dram_tensor("wk", (128, ND, ND, 128), F32,
                          kind="ExternalInput").ap()
    wv_d = nc.dram_tensor("wv", (128, ND, ND, 128), F32,
                          kind="ExternalInput").ap()
    # wo[p, dtile, o] = Wo[128*dtile+p, o]
    wo_d = nc.dram_tensor("wo", (128, ND, D), F32, kind="ExternalInput").ap()
    # biases host-packed [p, otile]
    bq_d = nc.dram_tensor("bq", (128, ND), F32, kind="ExternalInput").ap()
    bk_d = nc.dram_tensor("bk", (128, ND), F32, kind="ExternalInput").ap()
    bv_d = nc.dram_tensor("bv", (128, ND), F32, kind="ExternalInput").ap()
    bo_d = nc.dram_tensor("bo", (D,), F32, kind="ExternalInput").ap()
    gamma_d = nc.dram_tensor("gamma", (D,), F32, kind="ExternalInput").ap()
    beta_d = nc.dram_tensor("beta", (D,), F32, kind="ExternalInput").ap()
    y_d = nc.dram_tensor("y", (TQ, D), F32, kind="ExternalOutput").ap()

    def bcast_rows(src_row_ap, nrows):
        # replicate a [1, N] AP across nrows partitions (DMA only)
        return bass.AP(tensor=src_row_ap.tensor, offset=src_row_ap.offset,
                       ap=[[0, nrows]] + [list(d) for d in src_row_ap.ap[-1:]])

    with tile.TileContext(nc) as tc:
        from contextlib import ExitStack
        with ExitStack() as ctx:
            # ---- pools ----
            consts = ctx.enter_context(tc.tile_pool(name="consts", bufs=1))
            bigp = ctx.enter_context(tc.tile_pool(name="big", bufs=1))
            wslice = ctx.enter_context(tc.tile_pool(name="wslice", bufs=2))
            ktp = ctx.enter_context(tc.tile_pool(name="ktp", bufs=1))
            vts = ctx.enter_context(tc.tile_pool(name="vts", bufs=1))
            vaug = ctx.enter_context(tc.tile_pool(name="vaug", bufs=2))
            expp = ctx.enter_context(tc.tile_pool(name="expp", bufs=4))
            smallp = ctx.enter_context(tc.tile_pool(name="small", bufs=2))
            xnatp = ctx.enter_context(tc.tile_pool(name="xnat", bufs=2))
            ybufp = ctx.enter_context(tc.tile_pool(name="ybuf", bufs=1))

            ps_tr = ctx.enter_context(
                tc.tile_pool(name="ps_tr", bufs=2, space="PSUM"))
            ps_acc = ctx.enter_context(
                tc.tile_pool(name="ps_acc", bufs=2, space="PSUM"))
            ps_sc = ctx.enter_context(
                tc.tile_pool(name="ps_sc", bufs=2, space="PSUM"))

            # ---- constants ----
            ident = xnatp.tile([128, 128], F32, tag="xnat", name="ident")
            make_identity(nc, ident)
            eps_t = consts.tile([128, 1], F32)
            nc.vector.memset(eps_t, EPS)
            ones1 = consts.tile([128, 64], F32)
            nc.vector.memset(ones1, 1.0)
            rdt = F32 if sdt == F32 else F32R
            ones_r = consts.tile([128, 64], rdt)
            nc.vector.tensor_copy(out=ones_r, in_=ones1)
            ident_s = consts.tile([128, 128], sdt)
            nc.vector.tensor_copy(out=ident_s, in_=ident)
            ones_nt = consts.tile([128, NT, 1], F32)
            nc.vector.memset(ones_nt, 1.0)
            # per-partition biases [128, ND]: column j is bias[128j:128j+128]
            bq_t = consts.tile([128, ND], F32)
            nc.sync.dma_start(out=bq_t, in_=bq_d)
            bk_t = consts.tile([128, ND], F32)
            nc.sync.dma_start(out=bk_t, in_=bk_d)
            bv_t = consts.tile([128, ND], F32)
            nc.sync.dma_start(out=bv_t, in_=bv_d)

            # =========================================================
            # Phase A: load xT[p, d, t] = x[t, 128d+p] (pre-transposed on host)
            # =========================================================
            xT = bigp.tile([128, ND, S], sdt, tag="xT")
            xTd = [xT[:, d, :] for d in range(ND)]
            if sdt != F32:
                for d in range(ND):
                    for hf in range(2):
                        xtf = xnatp.tile([128, S // 2], F32, tag="xnat")
                        sl = slice(hf * (S // 2), (hf + 1) * (S // 2))
                        nc.sync.dma_start(out=xtf, in_=xbT_d[:, d, sl])
                        nc.vector.tensor_copy(out=xT[:, d, sl], in_=xtf)
            else:
                nc.sync.dma_start(out=xT, in_=xbT_d)

            # =========================================================
            # Phase B: QT[p, j, tq] = q[tq, 128j+p] for own tokens 0..TQ
            #   q = x @ Wq + bq   (scale folded into exp later)
            # =========================================================
            qT = bigp.tile([128, ND, TQ], sdt, tag="qT")
            for j in range(ND):
                wq_s = wslice.tile([128, ND, 128], sdt, tag="wsl")
                src = wq_d[:, j, :, :]
                if sdt != F32:
                    wq_f = xnatp.tile([128, ND, 128], F32, tag="xnat")
                    nc.sync.dma_start(out=wq_f, in_=src)
                    nc.vector.tensor_copy(out=wq_s, in_=wq_f)
                else:
                    nc.sync.dma_start(out=wq_s, in_=src)
                pq = ps_acc.tile([128, TQ], F32, tag="acc")
                for d in range(ND):
                    nc.tensor.matmul(pq, mm(wq_s[:, d, :]), mm(xTd[d][:, 0:TQ]),
                                     start=(d == 0), stop=(d == ND - 1))
                nc.vector.tensor_scalar_add(
                    out=qT[:, j, :], in0=pq, scalar1=bq_t[:, j:j + 1])

            # =========================================================
            # Phase C: per head-pair projections + attention
            # =========================================================
            outT = bigp.tile([128, ND, TQ], sdt, tag="outT")

            for p in range(NPAIR):
                # -- W slices for this pair --
                wk_s = wslice.tile([128, ND, 128], sdt, tag="wsl")
                wv_s = wslice.tile([128, ND, 128], sdt, tag="wsl")
                ksrc = wk_d[:, p, :, :]
                vsrc = wv_d[:, p, :, :]
                if sdt != F32:
                    wk_f = xnatp.tile([128, ND, 128], F32, tag="xnat")
                    wv_f = xnatp.tile([128, ND, 128], F32, tag="xnat")
                    nc.sync.dma_start(out=wk_f, in_=ksrc)
                    nc.sync.dma_start(out=wv_f, in_=vsrc)
                    nc.vector.tensor_copy(out=wk_s, in_=wk_f)
                    nc.vector.tensor_copy(out=wv_s, in_=wv_f)
                else:
                    nc.sync.dma_start(out=wk_s, in_=ksrc)
                    nc.sync.dma_start(out=wv_s, in_=vsrc)

                # -- K^T for pair: [128(dh pair), NT*128] --
                kT = ktp.tile([128, NT // 4, 512], sdt, tag="kT")
                for n in range(NT // 4):
                    pk = ps_acc.tile([128, 512], F32, tag="acc")
                    for d in range(ND):
                        nc.tensor.matmul(
                            pk, mm(wk_s[:, d, :]),
                            mm(xTd[d][:, 512 * n:512 * (n + 1)]),
                            start=(d == 0), stop=(d == ND - 1))
                    nc.vector.tensor_scalar_add(
                        out=kT[:, n, :], in0=pk, scalar1=bk_t[:, p:p + 1])

                # -- V for pair, via V^T then PE transpose, into V_aug --
                # V_aug[:, t, 65*he : 65*he+65] = [V_head | ones]
                va = vaug.tile([128, NT, 130], sdt, tag="va")
                nc.vector.tensor_copy(out=va[:, :, 64:65], in_=ones_nt)
                nc.vector.tensor_copy(out=va[:, :, 129:130], in_=ones_nt)
                for n in range(NT // 4):
                    pv = ps_acc.tile([128, 512], F32, tag="acc")
                    for d in range(ND):
                        nc.tensor.matmul(
                            pv, mm(wv_s[:, d, :]),
                            mm(xTd[d][:, 512 * n:512 * (n + 1)]),
                            start=(d == 0), stop=(d == ND - 1))
                    vts_t = vts.tile([128, 512], sdt, tag="vts")
                    nc.vector.tensor_scalar_add(
                        out=vts_t, in0=pv, scalar1=bv_t[:, p:p + 1])
                    for s in range(4):
                        t = 4 * n + s
                        pt = ps_tr.tile([128, 128], sdt, tag="tr")
                        nc.tensor.transpose(
                            pt, vts_t[:, 128 * s:128 * (s + 1)], ident_s)
                        nc.vector.tensor_copy(
                            out=va[:, t, 0:64], in_=pt[:, 0:64])
                        nc.vector.tensor_copy(
                            out=va[:, t, 65:129], in_=pt[:, 64:128])

                # -- attention: both heads interleaved in quarter-chunks so
                # PE always has matmuls queued while ScalarE runs exp --
                NCHUNK = 4
                TPC = NT // NCHUNK  # key tiles per chunk
                pav = [None, None]
                for he in range(2):
                    pav[he] = ps_acc.tile([128, TQ], F32, tag="acc",
                                          name=f"pav{he}")
                exq = {}
                for ch in range(NCHUNK):
                    for he in range(2):
                        ex = expp.tile([128, TPC, TQ], sdt, tag="ex",
                                       name=f"ex{he}_{ch}")
                        exq[(he, ch)] = ex
                        for g in range(TPC // 2):
                            psc = ps_sc.tile([128, 2, TQ], F32, tag="sc",
                                             name="psc")
                            for s2 in range(2):
                                t = ch * TPC + 2 * g + s2
                                lhs = kT[64 * he:64 * (he + 1),
                                         t // 4,
                                         128 * (t % 4):128 * (t % 4 + 1)]
                                rhs = qT[64 * he:64 * (he + 1), p, :]
                                nc.tensor.matmul(psc[:, s2, :], mm(lhs),
                                                 mm(rhs), start=True,
                                                 stop=True)
                            nc.scalar.activation(
                                out=ex[:, 2 * g:2 * g + 2, :], in_=psc,
                                func=mybir.ActivationFunctionType.Exp,
                                scale=SCALE)
                    for he in range(2):
                        ex = exq[(he, ch)]
                        for tt in range(TPC):
                            t = ch * TPC + tt
                            lhs = va[:, t, 65 * he:65 * he + 65]
                            nc.tensor.matmul(
                                pav[he][0:65, :], mm(lhs), mm(ex[:, tt, :]),
                                start=(t == 0), stop=(t == NT - 1))
                for he in range(2):
                    # normalize: out^T / denom. Broadcast the RAW denominator
                    # row across 64 partitions via a K=1 outer-product matmul
                    # (keeps the slow divide off the PE critical path), then
                    # one DVE divide.
                    dns = smallp.tile([128, TQ], rdt, tag="rcp", name="dns")
                    nc.vector.tensor_copy(out=dns[64:65, :],
                                          in_=pav[he][64:65, :])
                    rb = ps_tr.tile([64, TQ], F32, tag="tr", name="rb")
                    nc.tensor.matmul(rb, ones_r[64:65, :], dns[64:65, :],
                                     start=True, stop=True)
                    scr = smallp.tile([64, TQ], F32, tag="rbs", name="scr")
                    rrec = smallp.tile([64, TQ], F32, tag="rrec", name="rrec")
                    nc.vector.reciprocal_approx_accurate(
                        out=rrec, in_=rb, scratch=scr)
                    if he == 0:
                        nc.vector.tensor_mul(
                            out=outT[0:64, p, :],
                            in0=pav[he][0:64, :], in1=rrec[0:64, :])
                    else:
                        # compute at partitions 0:64, then DMA-shift to 64:128
                        tmp = smallp.tile([128, TQ], sdt, tag="otmp",
                                          name="tmp")
                        nc.vector.tensor_mul(
                            out=tmp[0:64, :],
                            in0=pav[he][0:64, :], in1=rrec[0:64, :])
                        nc.gpsimd.dma_start(
                            out=outT[64:128, p, :], in_=tmp[0:64, :])

            # =========================================================
            # Phase D: out-proj + residual + LayerNorm
            # =========================================================
            # bo/gamma/beta broadcast rows land in a dead expS slot
            lnc = expp.tile([128, 3, D], F32, tag="ex", name="lnc")
            nc.gpsimd.dma_start(out=lnc[:, 0, :], in_=bcast_rows(bo_d[None], 128))
            nc.gpsimd.dma_start(out=lnc[:, 1, :],
                                in_=bcast_rows(gamma_d[None], 128))
            nc.gpsimd.dma_start(out=lnc[:, 2, :],
                                in_=bcast_rows(beta_d[None], 128))
            bo_b, gamma_b, beta_b = lnc[:, 0, :], lnc[:, 1, :], lnc[:, 2, :]
            wo_t = bigp.tile([128, ND, D], sdt, tag="xT")  # reuse xT slot
            if sdt != F32:
                for d in range(ND):
                    wo_f = xnatp.tile([128, D], F32, tag="xnat")
                    nc.sync.dma_start(out=wo_f, in_=wo_d[:, d, :])
                    nc.vector.tensor_copy(out=wo_t[:, d, :], in_=wo_f)
            else:
                nc.sync.dma_start(out=wo_t, in_=wo_d)
            xq_t = bigp.tile([128, NTQ, D], F32, tag="qT")  # reuse qT slot
            nc.sync.dma_start(
                out=xq_t, in_=xq_d.rearrange("(i p) d -> p i d", p=128))

            for i in range(NTQ):
                po = ps_sc.tile([128, 2, 512], F32, tag="sc", name="po")
                for half in range(2):
                    dst = po[:, half, :]
                    for d in range(ND):
                        nc.tensor.matmul(
                            dst, mm(outT[:, d, 128 * i:128 * (i + 1)]),
                            mm(wo_t[:, d, 512 * half:512 * (half + 1)]),
                            start=(d == 0), stop=(d == ND - 1))
                ysb = ybufp.tile([128, D], F32, tag="ysb")
                pflat = po.rearrange("p a b -> p (a b)")
                # y = out + bo + x
                nc.vector.tensor_add(out=ysb, in0=pflat, in1=bo_b)
                nc.vector.tensor_add(out=ysb, in0=ysb, in1=xq_t[:, i, :])
                # LayerNorm
                stats = smallp.tile([128, 2, 6], F32, tag="stats")
                mv = smallp.tile([128, 2], F32, tag="mv")
                yv = ysb.rearrange("p (a b) -> p a b", a=2)
                for sg in range(2):
                    nc.vector.bn_stats(out=stats[:, sg, :], in_=yv[:, sg, :])
                nc.vector.bn_aggr(out=mv, in_=stats)
                sd = smallp.tile([128, 1], F32, tag="sd")
                nc.scalar.activation(out=sd, in_=mv[:, 1:2],
                                     func=mybir.ActivationFunctionType.Sqrt,
                                     bias=eps_t, scale=1.0)
                rstd = smallp.tile([128, 1], F32, tag="rstd")
                nc.vector.reciprocal(out=rstd, in_=sd)
                nc.vector.tensor_scalar(
                    out=ysb, in0=ysb, scalar1=mv[:, 0:1], scalar2=rstd,
                    op0=mybir.AluOpType.subtract, op1=mybir.AluOpType.mult)
                nc.vector.tensor_mul(out=ysb, in0=ysb, in1=gamma_b)
                nc.vector.tensor_add(out=ysb, in0=ysb, in1=beta_b)
                nc.sync.dma_start(out=y_d[128 * i:128 * (i + 1), :], in_=ysb)

    nc.compile()
    return nc


_PROGRAM_CACHE = {}


def _get_program(mm_mode: str):
    if mm_mode not in _PROGRAM_CACHE:
        _PROGRAM_CACHE[mm_mode] = _build_program(mm_mode)
    return _PROGRAM_CACHE[mm_mode]


def _pack_w(w):
    # [p, otile, dtile, c] = W[128*dtile+p, 128*otile+c], contiguous
    w = np.asarray(w, np.float32).reshape(ND, 128, ND, 128)
    return np.ascontiguousarray(w.transpose(1, 2, 0, 3))


def _pack_wo(w):
    # [p, dtile, o] = W[128*dtile+p, o]
    w = np.asarray(w, np.float32).reshape(ND, 128, D)
    return np.ascontiguousarray(w.transpose(1, 0, 2))


def _pack_b(b):
    # [p, otile] = b[128*otile+p]
    b = np.asarray(b, np.float32).reshape(ND, 128)
    return np.ascontiguousarray(b.transpose(1, 0))


def kernel(x, Wq, bq, Wk, bk, Wv, bv, Wo, bo, gamma, beta, _trace=False):
    x = np.asarray(x, dtype=np.float32)
    nc = _get_program(MM_MODE)

    wq_p, wk_p, wv_p = _pack_w(Wq), _pack_w(Wk), _pack_w(Wv)
    wo_p = _pack_wo(Wo)
    bq_p, bk_p, bv_p = _pack_b(bq), _pack_b(bk), _pack_b(bv)
    in_maps = []
    for c in range(N_CORES):
        b = c // CORES_PER_BATCH
        off = TQ * (c % CORES_PER_BATCH)
        xb = np.concatenate([x[b, off:], x[b, :off]], axis=0)
        xbT = np.ascontiguousarray(
            xb.T.reshape(ND, 128, S).transpose(1, 0, 2))
        in_maps.append({
            "xbT": xbT,
            "xq": np.ascontiguousarray(xb[0:TQ]),
            "wq": wq_p, "wk": wk_p, "wv": wv_p, "wo": wo_p,
            "bq": bq_p, "bk": bk_p, "bv": bv_p,
            "bo": np.asarray(bo, np.float32),
            "gamma": np.asarray(gamma, np.float32),
            "beta": np.asarray(beta, np.float32),
        })

    res = bass_utils.run_bass_kernel_spmd(
        nc, in_maps, list(range(N_CORES)), trace=_trace)

    y = np.empty((B, S, D), dtype=np.float32)
    for c in range(N_CORES):
        b = c // CORES_PER_BATCH
        off = TQ * (c % CORES_PER_BATCH)
        y[b, off:off + TQ] = res.results[c]["y"]

    kernel.last_exec_time_ns = res.exec_time_ns
    return y


kernel.last_exec_time_ns = None


# revision 38
# speedup vs baseline: 1.1550x; 1.0824x over previous
"""Trainium2 Bass kernel: MultiHeadAttention + residual + LayerNorm.

Problem shapes (hardcoded):
  x: (2, 2048, 1024) f32, 16 heads x 64 head_dim, scale = 64**-0.5
  y = LayerNorm(x + MHA(x))

Sharding: token-parallel over 8 cores. Core c handles batch b=c//4 and
query tokens [512*(c%4), 512*(c%4+1)) of that batch. Each core receives
its batch's full token sequence ROTATED so that its own 512 query tokens
are rows 0..511 (attention is permutation-invariant over keys, so K/V
token order does not matter). No cross-core collectives needed.
"""

import sys

sys.path.insert(0, "/opt/trn_rl_repo")

import numpy as np

import concourse.bass as bass
import concourse.bacc as bacc
import concourse.mybir as mybir
import concourse.tile as tile
from concourse import bass_utils
from concourse.masks import make_identity

# ---- problem constants ----
B = 2
S = 2048
D = 1024
H = 16
DH = 64
SCALE = DH ** -0.5
EPS = 1e-5

N_CORES = 8
CORES_PER_BATCH = N_CORES // B
TQ = S // CORES_PER_BATCH          # 512 query tokens per core
NT = S // 128                      # 16 key tiles of 128
ND = D // 128                      # 8 dim tiles of 128
NPAIR = H // 2                     # 8 head pairs
NTQ = TQ // 128                    # 4 query tiles

F32 = mybir.dt.float32
F32R = mybir.dt.float32r
BF16 = mybir.dt.bfloat16

# matmul compute dtype: "f32" (exact, 4 cyc/row), "f32r" (1 cyc/row, relaxed),
# "bf16" (1 cyc/row, bf16 storage)
MM_MODE = "f32r"


def _build_program(mm_mode: str):
    """Build the SPMD Bass program (same for all 8 cores)."""
    nc = bacc.Bacc("TRN2", target_bir_lowering=False, debug=False,
                   num_devices=N_CORES)

    # storage dtype for matmul operand tiles. float32r / bf16 rounding is
    # applied by the compute op that writes each tile (PSUM->SBUF copies).
    sdt = {"f32": F32, "f32r": F32R, "bf16": BF16}[mm_mode]

    def mm(ap):
        return ap

    # ---- DRAM I/O ----
    # x host-pretransposed: xbT[p, d, t] = x[t, 128d+p]; xq = x rows 0..TQ
    xbT_d = nc.dram_tensor("xbT", (128, ND, S), F32, kind="ExternalInput").ap()
    xq_d = nc.dram_tensor("xq", (TQ, D), F32, kind="ExternalInput").ap()
    # weights host-packed: wX[p, otile, dtile, c] = WX[128*dtile+p, 128*otile+c]
    wq_d = nc.dram_tensor("wq", (128, ND, ND, 128), F32,
                          kind="ExternalInput").ap()
    wk_d = nc.dram_tensor("wk", (128, ND, ND, 128), F32,
                          kind="ExternalInput").ap()
    wv_d = nc.dram_tensor("wv", (128, ND, ND, 128), F32,
                          kind="ExternalInput").ap()
    # wo[p, dtile, o] = Wo[128*dtile+p, o]
    wo_d = nc.dram_tensor("wo", (128, ND, D), F32, kind="ExternalInput").ap()
    # biases host-packed [p, otile]
    bq_d = nc.dram_tensor("bq", (128, ND), F32, kind="ExternalInput").ap()
    bk_d = nc.dram_tensor("bk", (128, ND), F32, kind="ExternalInput").ap()
    bv_d = nc.dram_tensor("bv", (128, ND), F32, kind="ExternalInput").ap()
    bo_d = nc.dram_tensor("bo", (D,), F32, kind="ExternalInput").ap()
    gamma_d = nc.dram_tensor("gamma", (D,), F32, kind="ExternalInput").ap()
    beta_d = nc.dram_tensor("beta", (D,), F32, kind="ExternalInput").ap()
    y_d = nc.dram_tensor("y", (TQ, D), F32, kind="ExternalOutput").ap()

    def bcast_rows(src_row_ap, nrows):
        # replicate a [1, N] AP across nrows partitions (DMA only)
        return bass.AP(tensor=src_row_ap.tensor, offset=src_row_ap.offset,
                       ap=[[0, nrows]] + [list(d) for d in src_row_ap.ap[-1:]])

    with tile.TileContext(nc) as tc:
        from contextlib import ExitStack
        with ExitStack() as ctx:
            # ---- pools ----
            consts = ctx.enter_context(tc.tile_pool(name="consts", bufs=1))
            bigp = ctx.enter_context(tc.tile_pool(name="big", bufs=1))
            wslice = ctx.enter_context(tc.tile_pool(name="wslice", bufs=2))
            ktp = ctx.enter_context(tc.tile_pool(name="ktp", bufs=1))
            vts = ctx.enter_context(tc.tile_pool(name="vts", bufs=1))
            vaug = ctx.enter_context(tc.tile_pool(name="vaug", bufs=2))
            expp = ctx.enter_context(tc.tile_pool(name="expp", bufs=4))
            smallp = ctx.enter_context(tc.tile_pool(name="small", bufs=2))
            xnatp = ctx.enter_context(tc.tile_pool(name="xnat", bufs=2))
            ybufp = ctx.enter_context(tc.tile_pool(name="ybuf", bufs=1))

            ps_tr = ctx.enter_context(
                tc.tile_pool(name="ps_tr", bufs=2, space="PSUM"))
            ps_acc = ctx.enter_context(
                tc.tile_pool(name="ps_acc", bufs=2, space="PSUM"))
            ps_sc = ctx.enter_context(
                tc.tile_pool(name="ps_sc", bufs=2, space="PSUM"))

            # =========================================================
            # Phase A: load xT[p, d, t] = x[t, 128d+p] (pre-transposed on host)
            # =========================================================
            xTd = []
            for d in range(ND):
                xt_d = bigp.tile([128, S], sdt, tag=f"xT{d}", name=f"xT{d}")
                xTd.append(xt_d)
                if sdt != F32:
                    for hf in range(2):
                        xtf = xnatp.tile([128, S // 2], F32, tag="xnat")
                        sl = slice(hf * (S // 2), (hf + 1) * (S // 2))
                        nc.sync.dma_start(out=xtf, in_=xbT_d[:, d, sl])
                        nc.vector.tensor_copy(out=xt_d[:, sl], in_=xtf)
                else:
                    nc.sync.dma_start(out=xt_d, in_=xbT_d[:, d, :])

            # ---- constants ----
            ident = xnatp.tile([128, 128], F32, tag="xnat", name="ident")
            make_identity(nc, ident)
            eps_t = consts.tile([128, 1], F32)
            nc.vector.memset(eps_t, EPS)
            ones1 = consts.tile([128, 64], F32)
            nc.vector.memset(ones1, 1.0)
            rdt = F32 if sdt == F32 else F32R
            ones_r = consts.tile([128, 64], rdt)
            nc.vector.tensor_copy(out=ones_r, in_=ones1)
            ident_s = consts.tile([128, 128], sdt)
            nc.vector.tensor_copy(out=ident_s, in_=ident)
            ones_nt = consts.tile([128, NT, 1], F32)
            nc.vector.memset(ones_nt, 1.0)
            # per-partition biases [128, ND]: column j is bias[128j:128j+128]
            bq_t = consts.tile([128, ND], F32)
            nc.sync.dma_start(out=bq_t, in_=bq_d)
            bk_t = consts.tile([128, ND], F32)
            nc.sync.dma_start(out=bk_t, in_=bk_d)
            bv_t = consts.tile([128, ND], F32)
            nc.sync.dma_start(out=bv_t, in_=bv_d)

            # =========================================================
            # Phase B: QT[p, j, tq] = q[tq, 128j+p] for own tokens 0..TQ
            #   q = x @ Wq + bq   (scale folded into exp later)
            # =========================================================
            qT = bigp.tile([128, ND, TQ], sdt, tag="qT")
            for j in range(ND):
                wq_s = wslice.tile([128, ND, 128], sdt, tag="wsl")
                src = wq_d[:, j, :, :]
                if sdt != F32:
                    wq_f = xnatp.tile([128, ND, 128], F32, tag="xnat")
                    nc.sync.dma_start(out=wq_f, in_=src)
                    nc.vector.tensor_copy(out=wq_s, in_=wq_f)
                else:
                    nc.sync.dma_start(out=wq_s, in_=src)
                pq = ps_acc.tile([128, TQ], F32, tag="acc")
                for d in range(ND):
                    nc.tensor.matmul(pq, mm(wq_s[:, d, :]), mm(xTd[d][:, 0:TQ]),
                                     start=(d == 0), stop=(d == ND - 1))
                nc.vector.tensor_scalar_add(
                    out=qT[:, j, :], in0=pq, scalar1=bq_t[:, j:j + 1])

            # =========================================================
            # Phase C: per head-pair projections + attention
            # =========================================================
            outT = bigp.tile([128, ND, TQ], sdt, tag="outT")

            for p in range(NPAIR):
                # -- W slices for this pair --
                wk_s = wslice.tile([128, ND, 128], sdt, tag="wsl")
                wv_s = wslice.tile([128, ND, 128], sdt, tag="wsl")
                ksrc = wk_d[:, p, :, :]
                vsrc = wv_d[:, p, :, :]
                if sdt != F32:
                    wk_f = xnatp.tile([128, ND, 128], F32, tag="xnat")
                    wv_f = xnatp.tile([128, ND, 128], F32, tag="xnat")
                    nc.sync.dma_start(out=wk_f, in_=ksrc)
                    nc.sync.dma_start(out=wv_f, in_=vsrc)
                    nc.vector.tensor_copy(out=wk_s, in_=wk_f)
                    nc.vector.tensor_copy(out=wv_s, in_=wv_f)
                else:
                    nc.sync.dma_start(out=wk_s, in_=ksrc)
                    nc.sync.dma_start(out=wv_s, in_=vsrc)

                # -- K^T for pair: [128(dh pair), NT*128] --
                kT = ktp.tile([128, NT // 4, 512], sdt, tag="kT")
                for n in range(NT // 4):
                    pk = ps_acc.tile([128, 512], F32, tag="acc")
                    for d in range(ND):
                        nc.tensor.matmul(
                            pk, mm(wk_s[:, d, :]),
                            mm(xTd[d][:, 512 * n:512 * (n + 1)]),
                            start=(d == 0), stop=(d == ND - 1))
                    nc.vector.tensor_scalar_add(
                        out=kT[:, n, :], in0=pk, scalar1=bk_t[:, p:p + 1])

                # -- V for pair, via V^T then PE transpose, into V_aug --
                # V_aug[:, t, 65*he : 65*he+65] = [V_head | ones]
                va = vaug.tile([128, NT, 130], sdt, tag="va")
                nc.vector.tensor_copy(out=va[:, :, 64:65], in_=ones_nt)
                nc.vector.tensor_copy(out=va[:, :, 129:130], in_=ones_nt)
                for n in range(NT // 4):
                    pv = ps_acc.tile([128, 512], F32, tag="acc")
                    for d in range(ND):
                        nc.tensor.matmul(
                            pv, mm(wv_s[:, d, :]),
                            mm(xTd[d][:, 512 * n:512 * (n + 1)]),
                            start=(d == 0), stop=(d == ND - 1))
                    vts_t = vts.tile([128, 512], sdt, tag="vts")
                    nc.vector.tensor_scalar_add(
                        out=vts_t, in0=pv, scalar1=bv_t[:, p:p + 1])
                    for s in range(4):
                        t = 4 * n + s
                        pt = ps_tr.tile([128, 128], sdt, tag="tr")
                        nc.tensor.transpose(
                            pt, vts_t[:, 128 * s:128 * (s + 1)], ident_s)
                        nc.vector.tensor_copy(
                            out=va[:, t, 0:64], in_=pt[:, 0:64])
                        nc.vector.tensor_copy(
                            out=va[:, t, 65:129], in_=pt[:, 64:128])

                # -- attention: both heads interleaved in quarter-chunks so
                # PE always has matmuls queued while ScalarE runs exp --
                NCHUNK = 4
                TPC = NT // NCHUNK  # key tiles per chunk
                pav = [None, None]
                for he in range(2):
                    pav[he] = ps_acc.tile([128, TQ], F32, tag="acc",
                                          name=f"pav{he}")
                exq = {}
                for ch in range(NCHUNK):
                    for he in range(2):
                        ex = expp.tile([128, TPC, TQ], sdt, tag="ex",
                                       name=f"ex{he}_{ch}")
                        exq[(he, ch)] = ex
                        for g in range(TPC // 2):
                            psc = ps_sc.tile([128, 2, TQ], F32, tag="sc",
                                             name="psc")
                            for s2 in range(2):
                                t = ch * TPC + 2 * g + s2
                                lhs = kT[64 * he:64 * (he + 1),
                                         t // 4,
                                         128 * (t % 4):128 * (t % 4 + 1)]
                                rhs = qT[64 * he:64 * (he + 1), p, :]
                                nc.tensor.matmul(psc[:, s2, :], mm(lhs),
                                                 mm(rhs), start=True,
                                                 stop=True)
                            nc.scalar.activation(
                                out=ex[:, 2 * g:2 * g + 2, :], in_=psc,
                                func=mybir.ActivationFunctionType.Exp,
                                scale=SCALE)
                    for he in range(2):
                        ex = exq[(he, ch)]
                        for tt in range(TPC):
                            t = ch * TPC + tt
                            lhs = va[:, t, 65 * he:65 * he + 65]
                            nc.tensor.matmul(
                                pav[he][0:65, :], mm(lhs), mm(ex[:, tt, :]),
                                start=(t == 0), stop=(t == NT - 1))
                for he in range(2):
                    # normalize: out^T / denom. Broadcast the RAW denominator
                    # row across 64 partitions via a K=1 outer-product matmul
                    # (keeps the slow divide off the PE critical path), then
                    # one DVE divide.
                    dns = smallp.tile([128, TQ], rdt, tag="rcp", name="dns")
                    nc.vector.tensor_copy(out=dns[64:65, :],
                                          in_=pav[he][64:65, :])
                    rb = ps_tr.tile([64, TQ], F32, tag="tr", name="rb")
                    nc.tensor.matmul(rb, ones_r[64:65, :], dns[64:65, :],
                                     start=True, stop=True)
                    scr = smallp.tile([64, TQ], F32, tag="rbs", name="scr")
                    rrec = smallp.tile([64, TQ], F32, tag="rrec", name="rrec")
                    nc.vector.reciprocal_approx_accurate(
                        out=rrec, in_=rb, scratch=scr)
                    if he == 0:
                        nc.vector.tensor_mul(
                            out=outT[0:64, p, :],
                            in0=pav[he][0:64, :], in1=rrec[0:64, :])
                    else:
                        # compute at partitions 0:64, then DMA-shift to 64:128
                        tmp = smallp.tile([128, TQ], sdt, tag="otmp",
                                          name="tmp")
                        nc.vector.tensor_mul(
                            out=tmp[0:64, :],
                            in0=pav[he][0:64, :], in1=rrec[0:64, :])
                        nc.gpsimd.dma_start(
                            out=outT[64:128, p, :], in_=tmp[0:64, :])

            # =========================================================
            # Phase D: out-proj + residual + LayerNorm
            # =========================================================
            # bo/gamma/beta broadcast rows land in a dead expS slot
            lnc = expp.tile([128, 3, D], F32, tag="ex", name="lnc")
            nc.gpsimd.dma_start(out=lnc[:, 0, :], in_=bcast_rows(bo_d[None], 128))
            nc.gpsimd.dma_start(out=lnc[:, 1, :],
                                in_=bcast_rows(gamma_d[None], 128))
            nc.gpsimd.dma_start(out=lnc[:, 2, :],
                                in_=bcast_rows(beta_d[None], 128))
            bo_b, gamma_b, beta_b = lnc[:, 0, :], lnc[:, 1, :], lnc[:, 2, :]
            # Wo reuses the xT slots (2 d-slices per 8KB slot)
            wo_td = []
            for d2 in range(ND // 2):
                wt = bigp.tile([128, 2, D], sdt, tag=f"xT{d2}", name=f"wo{d2}")
                wo_td.append(wt)
                for k2 in range(2):
                    d = 2 * d2 + k2
                    if sdt != F32:
                        wo_f = xnatp.tile([128, D], F32, tag="xnat")
                        nc.sync.dma_start(out=wo_f, in_=wo_d[:, d, :])
                        nc.vector.tensor_copy(out=wt[:, k2, :], in_=wo_f)
                    else:
                        nc.sync.dma_start(out=wt[:, k2, :], in_=wo_d[:, d, :])
            xq_t = bigp.tile([128, NTQ, D], F32, tag="qT")  # reuse qT slot
            nc.sync.dma_start(
                out=xq_t, in_=xq_d.rearrange("(i p) d -> p i d", p=128))

            for i in range(NTQ):
                po = ps_sc.tile([128, 2, 512], F32, tag="sc", name="po")
                for half in range(2):
                    dst = po[:, half, :]
                    for d in range(ND):
                        nc.tensor.matmul(
                            dst, mm(outT[:, d, 128 * i:128 * (i + 1)]),
                            mm(wo_td[d // 2][:, d % 2,
                                             512 * half:512 * (half + 1)]),
                            start=(d == 0), stop=(d == ND - 1))
                ysb = ybufp.tile([128, D], F32, tag="ysb")
                pflat = po.rearrange("p a b -> p (a b)")
                # y = out + bo + x
                nc.vector.tensor_add(out=ysb, in0=pflat, in1=bo_b)
                nc.vector.tensor_add(out=ysb, in0=ysb, in1=xq_t[:, i, :])
                # LayerNorm
                stats = smallp.tile([128, 2, 6], F32, tag="stats")
                mv = smallp.tile([128, 2], F32, tag="mv")
                yv = ysb.rearrange("p (a b) -> p a b", a=2)
                for sg in range(2):
                    nc.vector.bn_stats(out=stats[:, sg, :], in_=yv[:, sg, :])
                nc.vector.bn_aggr(out=mv, in_=stats)
                sd = smallp.tile([128, 1], F32, tag="sd")
                nc.scalar.activation(out=sd, in_=mv[:, 1:2],
                                     func=mybir.ActivationFunctionType.Sqrt,
                                     bias=eps_t, scale=1.0)
                rstd = smallp.tile([128, 1], F32, tag="rstd")
                nc.vector.reciprocal(out=rstd, in_=sd)
                nc.vector.tensor_scalar(
                    out=ysb, in0=ysb, scalar1=mv[:, 0:1], scalar2=rstd,
                    op0=mybir.AluOpType.subtract, op1=mybir.AluOpType.mult)
                nc.vector.tensor_mul(out=ysb, in0=ysb, in1=gamma_b)
                nc.vector.tensor_add(out=ysb, in0=ysb, in1=beta_b)
                nc.sync.dma_start(out=y_d[128 * i:128 * (i + 1), :], in_=ysb)

    nc.compile()
    return nc


_PROGRAM_CACHE = {}


def _get_program(mm_mode: str):
    if mm_mode not in _PROGRAM_CACHE:
        _PROGRAM_CACHE[mm_mode] = _build_program(mm_mode)
    return _PROGRAM_CACHE[mm_mode]


def _pack_w(w):
    # [p, otile, dtile, c] = W[128*dtile+p, 128*otile+c], contiguous
    w = np.asarray(w, np.float32).reshape(ND, 128, ND, 128)
    return np.ascontiguousarray(w.transpose(1, 2, 0, 3))


def _pack_wo(w):
    # [p, dtile, o] = W[128*dtile+p, o]
    w = np.asarray(w, np.float32).reshape(ND, 128, D)
    return np.ascontiguousarray(w.transpose(1, 0, 2))


def _pack_b(b):
    # [p, otile] = b[128*otile+p]
    b = np.asarray(b, np.float32).reshape(ND, 128)
    return np.ascontiguousarray(b.transpose(1, 0))


def kernel(x, Wq, bq, Wk, bk, Wv, bv, Wo, bo, gamma, beta, _trace=False):
    x = np.asarray(x, dtype=np.float32)
    nc = _get_program(MM_MODE)

    wq_p, wk_p, wv_p = _pack_w(Wq), _pack_w(Wk), _pack_w(Wv)
    wo_p = _pack_wo(Wo)
    bq_p, bk_p, bv_p = _pack_b(bq), _pack_b(bk), _pack_b(bv)
    in_maps = []
    for c in range(N_CORES):
        b = c // CORES_PER_BATCH
        off = TQ * (c % CORES_PER_BATCH)
        xb = np.concatenate([x[b, off:], x[b, :off]], axis=0)
        xbT = np.ascontiguousarray(
            xb.T.reshape(ND, 128, S).transpose(1, 0, 2))
        in_maps.append({
            "xbT": xbT,
            "xq": np.ascontiguousarray(xb[0:TQ]),
            "wq": wq_p, "wk": wk_p, "wv": wv_p, "wo": wo_p,
            "bq": bq_p, "bk": bk_p, "bv": bv_p,
            "bo": np.asarray(bo, np.float32),
            "gamma": np.asarray(gamma, np.float32),
            "beta": np.asarray(beta, np.float32),
        })

    res = bass_utils.run_bass_kernel_spmd(
        nc, in_maps, list(range(N_CORES)), trace=_trace)

    y = np.empty((B, S, D), dtype=np.float32)
    for c in range(N_CORES):
        b = c // CORES_PER_BATCH
        off = TQ * (c % CORES_PER_BATCH)
        y[b, off:off + TQ] = res.results[c]["y"]

    kernel.last_exec_time_ns = res.exec_time_ns
    return y


kernel.last_exec_time_ns = None
